# revision 1
# baseline (speedup 1.0000x reference)
"""Trainium2 Bass kernel for nn_CPCircuitLayer_63350767616542 (embedding_lookup).

Reference math:
    seq_emb = einsum("bsh,rh->bsr", hidden_states, W_seq)          # [B,S,R]
    hid_fac = hidden_embeddings * cp_weight[0][None, :]            # [H,R]
    out[b,n] = sum_r seq_emb[b, si[n], r] * hid_fac[hi[n], r]      # [B,N]
    return out.reshape(B, S, N // S)

all_indices is the row-major cartesian product of (seq_idx, hidden_idx), so the
gather is the identity and the whole layer collapses to a two-matmul chain:
    P = hidden_states @ W_seq.T @ hid_fac.T                        # [B,S,H]
A host-side fallback gather handles any non-cartesian index list.

Sharding: flatten (B,S) -> 2048 rows, shard rows across the 8 cores (256 rows
per core, data-parallel, no collectives). Each core computes
    TT = W_seq @ X_c.T                  ([64, 256], via lhsT = W_seq.T)
    O_c = X_c @ W_seq.T @ hid_fac.T     ([256, 512])
The host pre-packs per-core SBUF images (contraction dim on partitions, fully
contiguous DMA descriptors) and replicates the small rank-sized factors, then
concatenates the core outputs.

Device kernel (raw bass, hand-scheduled; matmuls in float32r, ~2e-4 rel err):
    SP:   input DMAs: [W image | xt k-chunks 0-1], [xt k-chunks 2-3], [hid_fac.T]
          then the two output DMAs
    Pool: memset of the PE warm-up tile
    PE:   warm-up dummy matmuls (lift the HAM clock gate during the DMA
          window), mm1 (4 accumulating matmuls, starting as soon as the first
          xt half lands), mm2 (one matmul per 128-row output chunk)
    DVE:  tt copy (PSUM->SBUF, split per output chunk), 2 output copies
"""

import os

import numpy as np

B, S, H, R = 2, 1024, 512, 64
N_CORES = 8
ROWS = B * S                      # 2048 flattened rows
RPC = ROWS // N_CORES             # 256 rows per core
KC = H // 128                     # 4 contraction chunks of 128
MC = RPC // 128                   # 2 output row chunks of 128
W_COLS = KC * R                   # 256 cols of the packed W image
HALF = (KC // 2) * RPC            # 512 xt cols per half-DMA

# matmul operand dtype: "f32" (exact, 4 cyc/row), "f32r" (~2e-4 rel err,
# full rate at N>=256), "bf16" (~3e-3 rel err, full rate + half DMA bytes)
MM_DTYPE = os.environ.get("BASS_MM_DTYPE", "f32r")
N_DUMMY = int(os.environ.get("BASS_N_DUMMY", "12"))

_cache = {}
LAST_RESULT = None                # BassKernelResults of the most recent run


def _np_in_dtype():
    if MM_DTYPE == "bf16":
        import ml_dtypes

        return ml_dtypes.bfloat16
    return np.float32


def _get_nc():
    key = ("nc", MM_DTYPE, N_DUMMY)
    if key in _cache:
        return _cache[key]

    import concourse.bass as bass
    import concourse.mybir as mybir

    f32 = mybir.dt.float32
    mmdt = {
        "f32": mybir.dt.float32,
        "f32r": mybir.dt.float32r,
        "bf16": mybir.dt.bfloat16,
    }[MM_DTYPE]

    nc = bass.Bass(
        "TRN2",
        target_bir_lowering=False,
        debug=False,
        num_devices=N_CORES,
    )

    xw_d = nc.dram_tensor("xw", [128, W_COLS + HALF], mmdt, kind="ExternalInput")
    xt23_d = nc.dram_tensor("xt23", [128, HALF], mmdt, kind="ExternalInput")
    h_d = nc.dram_tensor("h", [R, H], mmdt, kind="ExternalInput")
    out_d = nc.dram_tensor("out", [RPC, H], f32, kind="ExternalOutput")

    with (
        nc.sbuf_tensor([128, W_COLS + HALF], mmdt) as xw_sb,
        nc.sbuf_tensor([R, H], mmdt) as h_sb,
        nc.sbuf_tensor([128, HALF], mmdt) as xt23_sb,
        nc.sbuf_tensor([R, RPC], mmdt) as tt_sb,
        nc.sbuf_tensor([128, H], f32) as o0_sb,
        nc.sbuf_tensor([128, H], f32) as o1_sb,
        nc.sbuf_tensor([128, 256], mybir.dt.bfloat16) as dummy_sb,
        nc.psum_tensor([R, RPC], f32) as tt_ps,
        nc.psum_tensor([128, H], f32) as o0_ps,
        nc.psum_tensor([128, H], f32) as o1_ps,
        nc.psum_tensor([128, 256], f32) as dummy_ps,
        nc.semaphore("s_xt01") as s_xt01,
        nc.semaphore("s_xt2") as s_xt2,
        nc.semaphore("s_xt23") as s_xt23,
        nc.semaphore("s_h") as s_h,
        nc.semaphore("s_init") as s_init,
        nc.semaphore("s_pe") as s_pe,
        nc.semaphore("s_dve") as s_dve,
        nc.semaphore("s_oc") as s_oc,
        nc.semaphore("s_out") as s_out,
        nc.Block(no_gpsimd_drain=True) as block,
    ):
        o_sb = [o0_sb, o1_sb]
        o_ps = [o0_ps, o1_ps]
        w_sb = xw_sb[:, 0:W_COLS]

        def xt_chunk(k):
            if k < 2:
                return xw_sb[:, W_COLS + k * RPC : W_COLS + (k + 1) * RPC]
            return xt23_sb[:, (k - 2) * RPC : (k - 1) * RPC]

        @block.sync
        def _(sync):
            sync.dma_start(xw_sb[:], xw_d.ap()).then_inc(s_xt01, 16)
            sync.dma_start(xt23_sb[:, 0:RPC], xt23_d.ap()[:, 0:RPC]).then_inc(
                s_xt2, 16
            )
            sync.dma_start(
                xt23_sb[:, RPC : 2 * RPC], xt23_d.ap()[:, RPC : 2 * RPC]
            ).then_inc(s_xt23, 16)
            sync.dma_start(h_sb[:], h_d.ap()).then_inc(s_h, 16)

        @block.gpsimd
        def _(gpsimd):
            gpsimd.memset(dummy_sb[:], 0.0).then_inc(s_init, 1)

        @block.tensor
        def _(tensor):
            tensor.wait_ge(s_init, 1)
            for _ in range(N_DUMMY):
                nc.tensor.matmul(
                    dummy_ps[:], dummy_sb[:, 0:128], dummy_sb[:], start=True,
                    stop=True,
                )
            tensor.wait_ge(s_xt01, 16)
            for k in range(2):
                nc.tensor.matmul(
                    tt_ps[:],
                    w_sb[:, k * R : (k + 1) * R],
                    xt_chunk(k),
                    start=(k == 0),
                    stop=False,
                )
            for k in range(2, 4):
                tensor.wait_ge(s_xt2 if k == 2 else s_xt23, 16)
                mm = nc.tensor.matmul(
                    tt_ps[:],
                    w_sb[:, k * R : (k + 1) * R],
                    xt_chunk(k),
                    start=False,
                    stop=(k == 3),
                )
            mm.then_inc(s_pe, 1)
            tensor.wait_ge(s_h, 16)
            for m in range(MC):
                tensor.wait_ge(s_dve, m + 1)
                nc.tensor.matmul(
                    o_ps[m][:],
                    tt_sb[:, m * 128 : (m + 1) * 128],
                    h_sb[:],
                    start=True,
                    stop=True,
                ).then_inc(s_pe, 1)

        # Output pieces: one whole-bank piece per 128-row chunk. Finer splits
        # lose: each extra DMA adds 625ns HWDGE occupancy + 650ns first-byte
        # delay, pushing the last piece past the bandwidth-gated finish.
        out_pieces = [(0, 0, H), (1, 0, H)]

        @block.vector
        def _(vector):
            vector.wait_ge(s_pe, 1)
            # per-m halves so mm2[m] starts after its own half lands
            for m in range(MC):
                nc.vector.tensor_copy(
                    tt_sb[:, m * 128 : (m + 1) * 128],
                    tt_ps[:, m * 128 : (m + 1) * 128],
                ).then_inc(s_dve, 1)
            for m, c0, c1 in out_pieces:
                vector.wait_ge(s_pe, 2 + m)
                nc.vector.tensor_copy(
                    o_sb[m][:, c0:c1], o_ps[m][:, c0:c1]
                ).then_inc(s_oc, 1)

        @block.sync
        def _(sync):
            for q, (m, c0, c1) in enumerate(out_pieces):
                sync.wait_ge(s_oc, q + 1)
                sync.dma_start(
                    out_d.ap()[m * 128 : (m + 1) * 128, c0:c1],
                    o_sb[m][:, c0:c1],
                ).then_inc(s_out, 16)
            sync.wait_ge(s_out, 16 * len(out_pieces))

    # Drop the unused const-AP memsets bass emits unconditionally in its
    # preamble (the BIR verifier itself flags them as having no reader);
    # they serialize ~380ns on Pool ahead of the startup barrier.
    b0 = nc.m.functions[0].blocks[0]
    b0.instructions = [
        i
        for i in b0.instructions
        if not (
            type(i).__name__ == "InstMemset"
            and str(getattr(i.outs[0], "memref", "")).startswith("const-")
        )
    ]
    # Drop the exit all-engine-barrier semaphore ops: the SP stream already
    # ends on wait_ge(s_out) after the last output DMA receipt, so every
    # output byte is in HBM before any engine halts; the cross-engine
    # EVSEM handshake only aligns halt times (~260ns).
    for b in nc.m.functions[0].blocks:
        if str(getattr(b, "name", "")).endswith("_end"):
            b.instructions = [
                i
                for i in b.instructions
                if not (
                    type(i).__name__ == "InstEventSemaphore"
                    and str(i.name).startswith("aeb_barrier")
                )
            ]
    # Drop the startup all-engine barrier as well (~450ns): every
    # cross-engine dependency in this kernel is carried by its own
    # semaphores (s_init gates PE on Pool's memset; DMA sems gate all
    # consumers), and each engine's register preamble precedes its own
    # work within its own stream.
    b0.instructions = [
        i for i in b0.instructions if not str(i.name).startswith("barrier_")
    ]

    _cache[key] = nc
    return nc


def _pack_inputs(hidden_states, W_seq, hidden_embeddings, cp_weight):
    """Build the per-core packed SBUF images.

    xt image:   xt[c][p, k*RPC + n] = X[c*RPC + n, k*128 + p]
    W image:    w[p, k*R + r]       = W_seq[r, k*128 + p]
    h image:    h[r, j]             = hid_fac[j, r] = (hidden_embeddings * cp)[j, r]
    """
    ind = _np_in_dtype()
    X = hidden_states.reshape(ROWS, H)
    xt = (
        X.astype(ind)
        .reshape(N_CORES, RPC, KC, 128)  # [c, n, k, p]
        .transpose(0, 3, 2, 1)           # [c, p, k, n]
        .reshape(N_CORES, 128, KC * RPC)
    )
    w = (
        W_seq.astype(np.float32)
        .reshape(R, KC, 128)             # [r, k, p]
        .transpose(2, 1, 0)              # [p, k, r]
        .reshape(128, W_COLS)
        .astype(ind)
    )
    xw = np.ascontiguousarray(
        np.concatenate(
            [np.broadcast_to(w, (N_CORES, 128, W_COLS)), xt[:, :, :HALF]], axis=2
        )
    )                                    # [c, 128, W_COLS + HALF]
    xt23 = np.ascontiguousarray(xt[:, :, HALF:])
    h = np.ascontiguousarray(
        (hidden_embeddings * cp_weight[0][None, :]).T.astype(ind)
    )                                    # [64, 512]
    return xw, xt23, h


def _run_device(xw, xt23, h, trace=False, **run_kwargs):
    global LAST_RESULT
    from concourse.bass_utils import run_bass_kernel_spmd

    nc = _get_nc()
    in_maps = [{"xw": xw[c], "xt23": xt23[c], "h": h} for c in range(N_CORES)]
    res = run_bass_kernel_spmd(
        nc, in_maps, core_ids=list(range(N_CORES)), trace=trace, **run_kwargs
    )
    LAST_RESULT = res
    return np.concatenate([r["out"] for r in res.results], axis=0)  # [2048, 512]


def _host_reference(hidden_states, W_seq, hidden_embeddings, cp_weight):
    """Pure-numpy fallback (correct, host-only)."""
    hid_fac = hidden_embeddings * cp_weight[0][None, :]
    X = hidden_states.reshape(ROWS, H)
    return (X @ W_seq.T @ hid_fac.T).astype(np.float32)


def kernel(hidden_states, all_indices, W_seq, hidden_embeddings, cp_weight,
           trace=False, **run_kwargs):
    hidden_states = np.asarray(hidden_states, dtype=np.float32)
    W_seq = np.asarray(W_seq, dtype=np.float32)
    hidden_embeddings = np.asarray(hidden_embeddings, dtype=np.float32)
    cp_weight = np.asarray(cp_weight, dtype=np.float32)
    all_indices = np.asarray(all_indices)

    try:
        xw, xt23, h = _pack_inputs(
            hidden_states, W_seq, hidden_embeddings, cp_weight
        )
        Y = _run_device(xw, xt23, h, trace=trace, **run_kwargs)
    except Exception as e:  # device unavailable/wedged: stay correct on host
        import traceback

        traceback.print_exc()
        print(f"kernel: device path failed ({type(e).__name__}); "
              "falling back to host compute")
        Y = _host_reference(hidden_states, W_seq, hidden_embeddings, cp_weight)

    P = Y.reshape(B, S, H)

    n = all_indices.shape[0]
    si = all_indices[:, 0].astype(np.int64)
    hi = all_indices[:, 1].astype(np.int64)
    flat = si * H + hi
    if n == S * H and np.array_equal(flat, np.arange(S * H, dtype=np.int64)):
        return P  # cartesian-product indices: the gather is the identity
    return P.reshape(B, S * H)[:, flat].reshape(B, S, n // S)



# revision 12
# speedup vs baseline: 1.5282x; 1.5282x over previous
"""Trainium2 Bass kernel for nn_CPCircuitLayer_63350767616542 (embedding_lookup).

Reference math:
    seq_emb = einsum("bsh,rh->bsr", hidden_states, W_seq)          # [B,S,R]
    hid_fac = hidden_embeddings * cp_weight[0][None, :]            # [H,R]
    out[b,n] = sum_r seq_emb[b, si[n], r] * hid_fac[hi[n], r]      # [B,N]
    return out.reshape(B, S, N // S)

all_indices is the row-major cartesian product of (seq_idx, hidden_idx), so the
gather is the identity and the whole layer collapses to a two-matmul chain:
    P = hidden_states @ W_seq.T @ hid_fac.T                        # [B,S,H]
A host-side fallback gather handles any non-cartesian index list.

Sharding: flatten (B,S) -> 2048 rows, shard rows across the 8 cores (256 rows
per core, data-parallel, no collectives). Each core computes
    TT = W_seq @ X_c.T                  ([64, 256], via lhsT = W_seq.T)
    O_c = X_c @ W_seq.T @ hid_fac.T     ([256, 512])

Device kernel (raw bass, hand-scheduled, bf16 operands ~3e-3 rel err):
    SP:   three input DMAs (w+xt012 | xt3 | hid_fac.T), then the final
          wait on the output-DMA receipts.
    Pool: memset of the kv_writeback ctx-index tile, two PREPARE_ONLY
          kv_writeback descriptor preps (one per 128-row output chunk,
          prepared during the input-DMA window), then one trigger_dma per
          chunk as soon as that chunk's PSUM->SBUF copies land. The
          triggered SWDGE transfer needs no HWDGE slot and no DGE->DMA
          handoff delay, so the output tail is wait -> trigger -> SDMA.
    PE:   mm1 (4 accumulating matmuls into tt psum; k0-k2 start as soon
          as the first input DMA lands, k3 on the second), then one
          matmul per 128-row output chunk. No warm-up dummies: the PE
          p-state model gives full clock to a first matmul issued >3us
          into the kernel, and idle gaps here stay under the reset
          threshold.
    DVE:  tt half A copy (psum f32 -> sbuf bf16), then the left half of
          each output chunk's psum->sbuf copy.
    Act:  tt half B copy, then the right half of each output copy.
"""

import numpy as np

B, S, H, R = 2, 1024, 512, 64
N_CORES = 8
ROWS = B * S                      # 2048 flattened rows
RPC = ROWS // N_CORES             # 256 rows per core
KC = H // 128                     # 4 contraction chunks of 128
MC = RPC // 128                   # 2 output row chunks of 128
W_COLS = KC * R                   # 256 cols of the packed W image
GW = RPC // 2                     # 128 output rows per mm1 column group
XA_XT_COLS = KC * GW              # A-halves of all four xt chunks (512 cols)
TT_DVE = 80                       # DVE's share of each 128-col tt copy

_cache = {}
LAST_RESULT = None                # BassKernelResults of the most recent run


def _np_bf16():
    import ml_dtypes

    return ml_dtypes.bfloat16


def _fix_stride(ap_obj, dim, stride):
    """Set ap[dim] stride in place (unsqueeze() leaves stride 1, but
    kv_writeback derives batch_step / dho_stride from these entries)."""
    v = ap_obj.ap
    cnt = v[dim][1]
    v.pop(dim)
    v.insert(dim, (stride, cnt))
    return ap_obj


def _get_nc():
    if "nc" in _cache:
        return _cache["nc"]

    import bass_rust as _bass_rust
    import concourse.bass as bass
    import concourse.mybir as mybir
    from concourse.library_config import all_libraries, standard
    from concourse.library_overlay import lower_extended_insts

    f32 = mybir.dt.float32
    bf16 = mybir.dt.bfloat16
    i32 = mybir.dt.int32

    nc = bass.Bass(
        "TRN2",
        target_bir_lowering=False,
        debug=False,
        num_devices=N_CORES,
    )

    xa_d = nc.dram_tensor("xa", [128, W_COLS + XA_XT_COLS], bf16, kind="ExternalInput")
    xb_d = nc.dram_tensor("xb", [128, KC * GW], bf16, kind="ExternalInput")
    h_d = nc.dram_tensor("h", [R, H], bf16, kind="ExternalInput")
    out_d = nc.dram_tensor("out", [RPC, H], f32, kind="ExternalOutput")

    from contextlib import ExitStack

    with ExitStack() as stack:
        ec = stack.enter_context
        xa_sb = ec(nc.sbuf_tensor([128, W_COLS + XA_XT_COLS], bf16))
        xb_sb = ec(nc.sbuf_tensor([128, KC * GW], bf16))
        h_sb = ec(nc.sbuf_tensor([R, H], bf16))
        tt_sb = ec(nc.sbuf_tensor([R, RPC], bf16))
        o0_sb = ec(nc.sbuf_tensor([128, H], f32))
        o1_sb = ec(nc.sbuf_tensor([128, H], f32))
        ctx_sb = ec(nc.sbuf_tensor([128, 1], i32))
        tt_ps = ec(nc.psum_tensor([R, RPC], f32))
        o0_ps = ec(nc.psum_tensor([128, H], f32))
        o1_ps = ec(nc.psum_tensor([128, H], f32))
        s_in1 = ec(nc.semaphore("s_in1"))
        s_in2 = ec(nc.semaphore("s_in2"))
        s_h = ec(nc.semaphore("s_h"))
        s_mm1 = ec(nc.semaphore("s_mm1"))
        s_ttA = ec(nc.semaphore("s_ttA"))
        s_ttB = ec(nc.semaphore("s_ttB"))
        s_mm2 = ec(nc.semaphore("s_mm2"))
        s_oc0 = ec(nc.semaphore("s_oc0"))
        s_oc1 = ec(nc.semaphore("s_oc1"))
        s_prep = ec(nc.semaphore("s_prep"))
        s_out = ec(nc.semaphore("s_out"))
        block = ec(nc.Block(no_gpsimd_drain=True))
        o_sb = [o0_sb, o1_sb]
        o_ps = [o0_ps, o1_ps]
        s_oc = [s_oc0, s_oc1]
        w_sb = xa_sb[:, 0:W_COLS]

        def xt_chunk(g, k):
            # group g's 128-row half of contraction chunk k
            if g == 0:
                return xa_sb[:, W_COLS + k * GW : W_COLS + (k + 1) * GW]
            return xb_sb[:, k * GW : (k + 1) * GW]

        @block.sync
        def _(sync):
            sync.dma_start(xa_sb[:], xa_d.ap()).then_inc(s_in1, 16)
            sync.dma_start(xb_sb[:], xb_d.ap()).then_inc(s_in2, 16)
            sync.dma_start(h_sb[:], h_d.ap()).then_inc(s_h, 16)
            sync.wait_ge(s_out, 32)

        @block.gpsimd
        def _(gpsimd):
            gpsimd.memset(ctx_sb[:], 0)
            for m in range(MC):
                # out view [batch=1, dhi=128, dho=1, n_ctx=512] over this
                # chunk's 128 output rows; in view [128, 1, 1, 512].
                oap = _fix_stride(
                    _fix_stride(
                        out_d.ap()[m * 128 : (m + 1) * 128, :]
                        .unsqueeze(0)
                        .unsqueeze(2),
                        2,
                        H,
                    ),
                    0,
                    RPC * H,
                )
                iap = _fix_stride(
                    _fix_stride(o_sb[m][:].unsqueeze(1).unsqueeze(1), 1, H), 2, H
                )
                gpsimd.kv_writeback(
                    oap, iap, ctx_sb[:], prepare_only=True, sem=s_out
                ).then_inc(s_prep, 1)
            gpsimd.wait_ge(s_prep, MC)
            for m in range(MC):
                gpsimd.wait_ge(s_oc[m], 2)
                gpsimd.trigger_dma(1)

        @block.tensor
        def _(tensor):
            # mm1 split into two column groups (A = tt cols 0:128 from the
            # first input DMA, B = 128:256 from the second) so group A's
            # psum->sbuf copy and mm2[0] overlap group B's matmuls.
            def mm1(g, k, **kw):
                return nc.tensor.matmul(
                    tt_ps[:, g * 128 : (g + 1) * 128],
                    w_sb[:, k * R : (k + 1) * R],
                    xt_chunk(g, k),
                    **kw,
                )

            tensor.wait_ge(s_in1, 16)
            for k in range(4):
                mmA = mm1(0, k, start=(k == 0), stop=(k == 3))
            mmA.then_inc(s_mm1, 1)
            tensor.wait_ge(s_in2, 16)
            for k in range(4):
                mmB = mm1(1, k, start=(k == 0), stop=(k == 3))
            mmB.then_inc(s_mm1, 1)
            tensor.wait_ge(s_h, 16)
            for m, s_tt in enumerate((s_ttA, s_ttB)):
                tensor.wait_ge(s_tt, 2)
                nc.tensor.matmul(
                    o_ps[m][:],
                    tt_sb[:, m * 128 : (m + 1) * 128],
                    h_sb[:],
                    start=True,
                    stop=True,
                ).then_inc(s_mm2, 1)

        @block.vector
        def _(vector):
            # tt copies split DVE/Act, shares balanced to the engines'
            # cycle+access costs so both sems land together
            for g, s_tt in enumerate((s_ttA, s_ttB)):
                vector.wait_ge(s_mm1, g + 1)
                nc.vector.tensor_copy(
                    tt_sb[:, g * 128 : g * 128 + TT_DVE],
                    tt_ps[:, g * 128 : g * 128 + TT_DVE],
                ).then_inc(s_tt, 1)
            for m in range(MC):
                vector.wait_ge(s_mm2, m + 1)
                nc.vector.tensor_copy(
                    o_sb[m][:, 0 : H // 2], o_ps[m][:, 0 : H // 2]
                ).then_inc(s_oc[m], 1)

        @block.scalar
        def _(scalar):
            for g, s_tt in enumerate((s_ttA, s_ttB)):
                scalar.wait_ge(s_mm1, g + 1)
                nc.scalar.copy(
                    tt_sb[:, g * 128 + TT_DVE : (g + 1) * 128],
                    tt_ps[:, g * 128 + TT_DVE : (g + 1) * 128],
                ).then_inc(s_tt, 1)
            for m in range(MC):
                scalar.wait_ge(s_mm2, m + 1)
                nc.scalar.copy(
                    o_sb[m][:, H // 2 : H], o_ps[m][:, H // 2 : H]
                ).then_inc(s_oc[m], 1)

    # The Ant SWDGE instructions (kv_writeback) run on the GpSimd Q7 and
    # need their ucode library loaded; Bacc inserts these LOAD_LIBs in its
    # own pipeline, raw bass must do it explicitly. lower_extended_insts
    # then populates .instr bytes for the extended InstISA subclasses
    # (trigger_dma), without which walrus codegen fails "ISA wrong length".
    inst_type_to_lib_mask = {}
    for lib in all_libraries:
        for inst_type in lib.instructions:
            inst_type_to_lib_mask[inst_type] = inst_type_to_lib_mask.get(
                inst_type, 0
            ) | (1 << lib.index)
    _bass_rust.insert_library_loads(
        nc, inst_type_to_lib_mask, len(all_libraries), standard.index
    )
    lower_extended_insts(nc)

    # Drop the unused const-AP memsets bass emits unconditionally in its
    # preamble (the BIR verifier itself flags them as having no reader);
    # they serialize ~380ns on Pool ahead of the startup barrier.
    b0 = nc.m.functions[0].blocks[0]
    b0.instructions = [
        i
        for i in b0.instructions
        if not (
            type(i).__name__ == "InstMemset"
            and str(getattr(i.outs[0], "memref", "")).startswith("const-")
        )
    ]
    # Drop the exit all-engine-barrier semaphore ops: the SP stream already
    # ends on wait_ge(s_out) after the last output DMA receipt, so every
    # output byte is in HBM before any engine halts; the cross-engine
    # EVSEM handshake only aligns halt times (~260ns).
    for b in nc.m.functions[0].blocks:
        if str(getattr(b, "name", "")).endswith("_end"):
            b.instructions = [
                i
                for i in b.instructions
                if not (
                    type(i).__name__ == "InstEventSemaphore"
                    and str(i.name).startswith("aeb_barrier")
                )
            ]
    # Drop the startup all-engine barrier as well (~450ns): every
    # cross-engine dependency in this kernel is carried by its own
    # semaphores, and each engine's register preamble precedes its own
    # work within its own stream.
    b0.instructions = [
        i for i in b0.instructions if not str(i.name).startswith("barrier_")
    ]

    _cache["nc"] = nc
    return nc


def _pack_inputs(hidden_states, W_seq, hidden_embeddings, cp_weight):
    """Build the per-core packed SBUF images (bf16).

    xt image:   xt[c][p, g, k, n] = X[c*RPC + g*GW + n, k*128 + p]
    W image:    w[p, k*R + r]     = W_seq[r, k*128 + p]
    h image:    h[r, j]           = hid_fac[j, r] = (hidden_embeddings * cp)[j, r]
    xa = [w | group-A halves of the four xt chunks], xb = group-B halves.
    """
    bf16 = _np_bf16()
    X = hidden_states.reshape(ROWS, H)
    xt = (
        X.astype(bf16)
        .reshape(N_CORES, MC, GW, KC, 128)  # [c, g, n, k, p]
        .transpose(0, 4, 1, 3, 2)           # [c, p, g, k, n]
        .reshape(N_CORES, 128, MC, KC * GW)
    )
    w = (
        W_seq.astype(np.float32)
        .reshape(R, KC, 128)             # [r, k, p]
        .transpose(2, 1, 0)              # [p, k, r]
        .reshape(128, W_COLS)
        .astype(bf16)
    )
    xa = np.ascontiguousarray(
        np.concatenate(
            [np.broadcast_to(w, (N_CORES, 128, W_COLS)), xt[:, :, 0]], axis=2
        )
    )                                    # [c, 128, W_COLS + KC*GW]
    xb = np.ascontiguousarray(xt[:, :, 1])
    h = np.ascontiguousarray(
        (hidden_embeddings * cp_weight[0][None, :]).T.astype(bf16)
    )                                    # [64, 512]
    return xa, xb, h


def _run_device(xa, xb, h, trace=False, **run_kwargs):
    global LAST_RESULT
    from concourse.bass_utils import run_bass_kernel_spmd

    nc = _get_nc()
    in_maps = [{"xa": xa[c], "xb": xb[c], "h": h} for c in range(N_CORES)]
    res = run_bass_kernel_spmd(
        nc, in_maps, core_ids=list(range(N_CORES)), trace=trace, **run_kwargs
    )
    LAST_RESULT = res
    return np.concatenate([r["out"] for r in res.results], axis=0)  # [2048, 512]


def _host_reference(hidden_states, W_seq, hidden_embeddings, cp_weight):
    """Pure-numpy fallback (correct, host-only)."""
    hid_fac = hidden_embeddings * cp_weight[0][None, :]
    X = hidden_states.reshape(ROWS, H)
    return (X @ W_seq.T @ hid_fac.T).astype(np.float32)


def kernel(hidden_states, all_indices, W_seq, hidden_embeddings, cp_weight,
           trace=False, **run_kwargs):
    hidden_states = np.asarray(hidden_states, dtype=np.float32)
    W_seq = np.asarray(W_seq, dtype=np.float32)
    hidden_embeddings = np.asarray(hidden_embeddings, dtype=np.float32)
    cp_weight = np.asarray(cp_weight, dtype=np.float32)
    all_indices = np.asarray(all_indices)

    try:
        xa, xb, h = _pack_inputs(
            hidden_states, W_seq, hidden_embeddings, cp_weight
        )
        Y = _run_device(xa, xb, h, trace=trace, **run_kwargs)
    except Exception as e:  # device unavailable/wedged: stay correct on host
        import traceback

        traceback.print_exc()
        print(f"kernel: device path failed ({type(e).__name__}); "
              "falling back to host compute")
        Y = _host_reference(hidden_states, W_seq, hidden_embeddings, cp_weight)

    P = Y.reshape(B, S, H)

    n = all_indices.shape[0]
    si = all_indices[:, 0].astype(np.int64)
    hi = all_indices[:, 1].astype(np.int64)
    flat = si * H + hi
    if n == S * H and np.array_equal(flat, np.arange(S * H, dtype=np.int64)):
        return P  # cartesian-product indices: the gather is the identity
    return P.reshape(B, S * H)[:, flat].reshape(B, S, n // S)
